# revision 1
# baseline (speedup 1.0000x reference)
"""Trainium2 Bass kernel for causal multi-head attention with RoPE.

Full-input contract: kernel(**inputs) takes the unsharded tensors and
returns the full [B, S, D] output. Internally the work is sharded over
8 NeuronCores: cores 0-3 compute batch 0, cores 4-7 batch 1; within a
batch group each core owns 4 of the 16 heads (tensor-parallel over
heads). Each core computes its partial output-projection contribution
[S, D]; the host sums the 4 partials per batch and adds the biases
that commute with attention (wo_b, and wv_b which passes through the
softmax untouched because attention weights sum to 1).

Matmuls run in float32r (hardware rounds operands to ~11 mantissa
bits, fp32 accumulate in PSUM) at 4x the fp32 rate.
"""

import os
import sys

sys.path.insert(0, "/opt/trn_rl_repo")

import numpy as np

B = 2
S = 2048
D = 2048
H = 16
DK = 128
N_CORES = 8
HPC = 4          # heads per core
E = HPC * DK     # 512: per-core slice of the model dim
AN = 256         # phase-A sequence chunk (moving free dim for Q/K)
SC = 512         # attention query chunk (moving free dim)
KO = D // 128    # contraction chunks for the projections
NJ = S // 128    # key chunks
NI = S // SC     # query chunks
ISQRT_DK = 1.0 / np.sqrt(DK)

_CACHE = {}

last_exec_time_ns = None
last_results = None


def _build_program():
    import concourse.mybir as mybir
    import concourse.tile as tile
    from concourse import bacc

    dt = mybir.dt
    F32 = dt.float32
    F32R = dt.float32r
    AF = mybir.ActivationFunctionType

    nc = bacc.Bacc(None, target_bir_lowering=False, debug=True)

    xT = nc.dram_tensor("xT", [D, S], F32R, kind="ExternalInput")
    wqT = nc.dram_tensor("wqT", [D, E], F32R, kind="ExternalInput")
    wkT = nc.dram_tensor("wkT", [D, E], F32R, kind="ExternalInput")
    wvT = nc.dram_tensor("wvT", [D, E], F32R, kind="ExternalInput")
    woT = nc.dram_tensor("woT", [E, D], F32R, kind="ExternalInput")
    bq = nc.dram_tensor("bq", [HPC, DK], F32, kind="ExternalInput")
    bk = nc.dram_tensor("bk", [HPC, DK], F32, kind="ExternalInput")
    cc2 = nc.dram_tensor("cc2", [DK, S], F32R, kind="ExternalInput")
    sss = nc.dram_tensor("sss", [DK, S], F32R, kind="ExternalInput")
    masks = nc.dram_tensor("masks", [HPC, 128, SC], F32R, kind="ExternalInput")
    ones = nc.dram_tensor("ones", [128, 128], F32R, kind="ExternalInput")
    out = nc.dram_tensor("out", [S, D], F32, kind="ExternalOutput")

    with tile.TileContext(nc) as tc:
        with (
            tc.tile_pool(name="dram", bufs=1, space="DRAM") as dpool,
            tc.tile_pool(name="const", bufs=1) as cpool,
        ):
            q_d = dpool.tile([HPC, DK, S], F32R, name="q_d")
            k_d = dpool.tile([HPC, DK, S], F32R, name="k_d")

            bq_sb = cpool.tile([DK, HPC], F32, name="bq_sb")
            nc.sync.dma_start(bq_sb[:], bq[:].rearrange("h d -> d h"))
            bk_sb = cpool.tile([DK, HPC], F32, name="bk_sb")
            nc.sync.dma_start(bk_sb[:], bk[:].rearrange("h d -> d h"))
            cc2_sb = cpool.tile([DK, S], F32R, name="cc2_sb")
            nc.gpsimd.dma_start(cc2_sb[:], cc2[:])
            sss_sb = cpool.tile([DK, S], F32R, name="sss_sb")
            nc.gpsimd.dma_start(sss_sb[:], sss[:])

            # V stays resident in SBUF from the projection through attention
            vres_ctx = tc.tile_pool(name="vres", bufs=1)
            vrpool = vres_ctx.__enter__()
            vt_all = vrpool.tile([128, NJ, E], F32R, name="vt_all")

            # ---------- Phase A: Q/K/V projections (+ RoPE on Q/K) ----------
            with (
                tc.tile_pool(name="aw", bufs=1) as awpool,
                tc.tile_pool(name="ax", bufs=2) as axpool,
                tc.tile_pool(name="ast", bufs=2) as astpool,
                tc.tile_pool(name="aso", bufs=3) as asopool,
                tc.tile_pool(name="aps", bufs=2, space="PSUM") as apspool,
            ):
                # per-k-chunk DMAs so the first matmuls start as soon as the
                # first 256KB pieces land (instead of after whole-tensor DMAs)
                def load_xn(n):
                    xn = axpool.tile([128, KO, AN], F32R, tag="xn", name=f"xn{n}")
                    for g in range(4):
                        nc.sync.dma_start(
                            xn[:, g * 4 : (g + 1) * 4, :],
                            xT[
                                g * 512 : (g + 1) * 512,
                                n * AN : (n + 1) * AN,
                            ].rearrange("(ko p) s -> p ko s", p=128),
                        )
                    return xn

                # strictly need-ordered input stream on one queue: the first
                # Q chain consumes (wq, x0) k-group pairs, then K needs wk,
                # then V needs wv; later x chunks stream behind
                wq_sb = awpool.tile([128, KO, E], F32R, name="wq_sb")
                wk_sb = awpool.tile([128, KO, E], F32R, name="wk_sb")
                wv_sb = awpool.tile([128, KO, E], F32R, name="wv_sb")
                xn_next = axpool.tile([128, KO, AN], F32R, tag="xn", name="xn0")
                for g in range(4):
                    nc.sync.dma_start(
                        wq_sb[:, g * 4 : (g + 1) * 4, :],
                        wqT[g * 512 : (g + 1) * 512, :].rearrange(
                            "(ko p) m -> p ko m", p=128
                        ),
                    )
                    nc.sync.dma_start(
                        xn_next[:, g * 4 : (g + 1) * 4, :],
                        xT[g * 512 : (g + 1) * 512, 0:AN].rearrange(
                            "(ko p) s -> p ko s", p=128
                        ),
                    )
                for wsb, wdram in ((wk_sb, wkT), (wv_sb, wvT)):
                    for g in range(4):
                        nc.sync.dma_start(
                            wsb[:, g * 4 : (g + 1) * 4, :],
                            wdram[g * 512 : (g + 1) * 512, :].rearrange(
                                "(ko p) m -> p ko m", p=128
                            ),
                        )

                for n in range(S // AN):
                    xn = xn_next
                    if n + 1 < S // AN:
                        xn_next = load_xn(n + 1)
                    nsl = slice(n * AN, (n + 1) * AN)
                    # Q and K: out[d, s], then bias + RoPE here (DVE is idle
                    # during the projections; keeps attention phase lean).
                    # First chunk runs k-outer so the PE consumes each weight/x
                    # k-group as it arrives instead of stalling mid-chain.
                    for wsb, bsb, dst in ((wq_sb, bq_sb, q_d), (wk_sb, bk_sb, k_d)):
                        if n == 0:
                            pqs = [
                                apspool.tile(
                                    [128, AN], F32, tag="pqk0", name=f"pq0_{m}"
                                )
                                for m in range(HPC)
                            ]
                            for k in range(KO):
                                for m in range(HPC):
                                    nc.tensor.matmul(
                                        pqs[m][:],
                                        wsb[:, k, m * DK : (m + 1) * DK],
                                        xn[:, k, :],
                                        start=(k == 0),
                                        stop=(k == KO - 1),
                                    )
                        for m in range(HPC):
                            if n == 0:
                                pq = pqs[m]
                            else:
                                pq = apspool.tile([128, AN], F32, tag="pqk")
                                for k in range(KO):
                                    nc.tensor.matmul(
                                        pq[:],
                                        wsb[:, k, m * DK : (m + 1) * DK],
                                        xn[:, k, :],
                                        start=(k == 0),
                                        stop=(k == KO - 1),
                                    )
                            st0 = astpool.tile([128, AN], F32, tag="qkst0")
                            nc.scalar.activation(
                                st0[:], pq[:], AF.Identity, bias=bsb[:, m : m + 1]
                            )
                            # RoPE: d-rows are packed [even; odd] per head, so
                            # rotate pairs are partition r <-> r+64
                            sw = astpool.tile([128, AN], F32, tag="qksw")
                            nc.vector.tensor_copy(sw[0:64, :], st0[64:128, :])
                            nc.vector.tensor_copy(sw[64:128, :], st0[0:64, :])
                            rot = asopool.tile([128, AN], F32R, tag="stout", name="rot")
                            nc.vector.tensor_mul(rot[:], st0[:], cc2_sb[:, nsl])
                            nc.vector.tensor_mul(sw[:], sw[:], sss_sb[:, nsl])
                            nc.vector.tensor_add(rot[:], rot[:], sw[:])
                            nc.scalar.dma_start(dst[m, :, nsl], rot[:])
                    # V: out[s, d] with s on partitions (natural for P@V)
                    for jj in range(AN // 128):
                        pv = apspool.tile([128, E], F32, tag="pv")
                        for k in range(KO):
                            nc.tensor.matmul(
                                pv[:],
                                xn[:, k, jj * 128 : (jj + 1) * 128],
                                wv_sb[:, k, :],
                                start=(k == 0),
                                stop=(k == KO - 1),
                            )
                        jc_g = (n * AN) // 128 + jj
                        nc.vector.tensor_copy(vt_all[:, jc_g, :], pv[:])

            # ---------- Phase B: causal attention per head ----------
            bc_ctx = tc.tile_pool(name="bconst", bufs=1)
            bcpool = bc_ctx.__enter__()
            ao_ctx = tc.tile_pool(name="ao", bufs=1)
            aopool = ao_ctx.__enter__()
            cw_ctx = tc.tile_pool(name="cw", bufs=1)
            cwpool = cw_ctx.__enter__()
            mask_sb = bcpool.tile([128, HPC, SC], F32R, name="mask_sb")
            nc.sync.dma_start(mask_sb[:], masks[:].rearrange("t p c -> p t c"))
            ones_sb = bcpool.tile([128, 128], F32R, name="ones_sb")
            nc.sync.dma_start(ones_sb[:], ones[:])

            ao_tiles = []
            wo_sb = cwpool.tile([128, HPC, D], F32R, name="wo_sb")
            with (
                tc.tile_pool(name="bkv", bufs=2) as bkv,
                tc.tile_pool(name="bp", bufs=6) as bp,
                tc.tile_pool(name="bli", bufs=2) as bli,
                tc.tile_pool(name="bps_s", bufs=4, space="PSUM") as bps_s,
                tc.tile_pool(name="bps_o", bufs=2, space="PSUM") as bps_o,
                tc.tile_pool(name="bps_l", bufs=2, space="PSUM") as bps_l,
            ):
                for h0 in range(HPC):
                    ktr = bkv.tile([DK, S], F32R, tag="ktr")
                    for si in range(NI):
                        sl = slice(si * SC, (si + 1) * SC)
                        nc.sync.dma_start(ktr[:, sl], k_d[h0][:, sl])
                    qtr = bkv.tile([DK, S], F32R, tag="qtr")
                    for si in range(NI):
                        sl = slice(si * SC, (si + 1) * SC)
                        nc.sync.dma_start(qtr[:, sl], q_d[h0][:, sl])
                    if h0 == 0:
                        # prefetch the output-projection weights during B
                        nc.sync.dma_start(
                            wo_sb[:],
                            woT[:].rearrange("(ec p) f -> p ec f", p=128),
                        )

                    ao_t = aopool.tile([DK, S], F32R, name=f"ao_{h0}")
                    ao_tiles.append(ao_t)

                    for ic in range(NI):
                        po = bps_o.tile([128, SC], F32, tag="po")
                        pl = bps_l.tile([128, SC], F32, tag="pl")
                        njc = 4 * ic + 4
                        i0 = ic * SC

                        def emit_pv(p, jc, cs):
                            nc.tensor.matmul(
                                po[:, cs:],
                                vt_all[:, jc, h0 * DK : (h0 + 1) * DK],
                                p[:, cs:],
                                start=(jc == 0),
                                stop=(jc == njc - 1),
                            )
                            nc.tensor.matmul(
                                pl[:, cs:],
                                ones_sb[:],
                                p[:, cs:],
                                start=(jc == 0),
                                stop=(jc == njc - 1),
                            )

                        pending = []
                        for jc in range(njc):
                            t = jc - 4 * ic  # >=0 on the causal diagonal band
                            cs = 128 * t if t >= 0 else 0
                            ps = bps_s.tile([128, SC], F32, tag="ps")
                            nc.tensor.matmul(
                                ps[:, cs:],
                                ktr[:, jc * 128 : (jc + 1) * 128],
                                qtr[:, i0 + cs : i0 + SC],
                                start=True,
                                stop=True,
                            )
                            p = bp.tile([128, SC], F32R, tag="p")
                            nc.scalar.activation(
                                p[:, cs:], ps[:, cs:], AF.Exp, scale=float(ISQRT_DK)
                            )
                            if t >= 0:
                                nc.vector.tensor_mul(
                                    p[:, cs : cs + 128],
                                    p[:, cs : cs + 128],
                                    mask_sb[:, t, cs : cs + 128],
                                )
                            # software pipeline: scores run up to two tiles
                            # ahead of the P@V / row-sum matmuls so the ACT
                            # exp latency stays off the tensor-engine path
                            pending.append((p, jc, cs))
                            if len(pending) > 2:
                                emit_pv(*pending.pop(0))
                        for it in pending:
                            emit_pv(*it)

                        li = bli.tile([128, SC], F32, tag="li")
                        nc.vector.reciprocal_approx_fast(li[:], pl[:])
                        nc.vector.tensor_mul(
                            ao_t[:, i0 : i0 + SC], po[:], li[:]
                        )

            # ---------- Phase C: output projection (partial sum) ----------
            with (
                tc.tile_pool(name="cst", bufs=6) as cst,
                tc.tile_pool(name="cps", bufs=8, space="PSUM") as cps,
            ):
                for ii in range(S // 128):
                    pcs = [
                        cps.tile([128, 512], F32, tag="pc", name=f"pc_{ii}_{fc}")
                        for fc in range(4)
                    ]
                    for ec in range(HPC):
                        for fc in range(4):
                            nc.tensor.matmul(
                                pcs[fc][:],
                                ao_tiles[ec][:, ii * 128 : (ii + 1) * 128],
                                wo_sb[:, ec, fc * 512 : (fc + 1) * 512],
                                start=(ec == 0),
                                stop=(ec == HPC - 1),
                            )
                    for fc in range(4):
                        ob = cst.tile([128, 512], F32, tag="ob")
                        if fc % 2 == 0:
                            nc.vector.tensor_copy(ob[:], pcs[fc][:])
                        else:
                            nc.scalar.activation(ob[:], pcs[fc][:], AF.Copy)
                        nc.sync.dma_start(
                            out[ii * 128 : (ii + 1) * 128, fc * 512 : (fc + 1) * 512],
                            ob[:],
                        )

            cw_ctx.__exit__(None, None, None)
            ao_ctx.__exit__(None, None, None)
            bc_ctx.__exit__(None, None, None)
            vres_ctx.__exit__(None, None, None)

    nc.compile()
    return nc


def _rope_tables():
    inv_freq = 1.0 / (10000.0 ** (np.arange(0, DK, 2, dtype=np.float64) / DK))
    pos = np.arange(S, dtype=np.float64)
    freqs = pos[:, None] * inv_freq[None, :]  # [S, DK/2]
    cos_t = np.cos(freqs).T.astype(np.float32)  # [64, S]
    sin_t = np.sin(freqs).T.astype(np.float32)
    cc2 = np.ascontiguousarray(np.concatenate([cos_t, cos_t], axis=0))
    sss = np.ascontiguousarray(np.concatenate([-sin_t, sin_t], axis=0))
    return cc2, sss


def kernel(
    x, wq_w, wq_b, wk_w, wk_b, wv_w, wv_b, wo_w, wo_b
) -> np.ndarray:
    global last_exec_time_ns, last_results
    from concourse.bass_utils import run_bass_kernel_spmd

    if "nc" not in _CACHE:
        _CACHE["nc"] = _build_program()
    nc = _CACHE["nc"]

    x = np.asarray(x, dtype=np.float32)
    wq_w = np.asarray(wq_w, dtype=np.float32)
    wk_w = np.asarray(wk_w, dtype=np.float32)
    wv_w = np.asarray(wv_w, dtype=np.float32)
    wo_w = np.asarray(wo_w, dtype=np.float32)
    wq_b = np.asarray(wq_b, dtype=np.float32)
    wk_b = np.asarray(wk_b, dtype=np.float32)
    wv_b = np.asarray(wv_b, dtype=np.float32)
    wo_b = np.asarray(wo_b, dtype=np.float32)

    cc2, sss = _rope_tables()
    r_idx = np.arange(128)[:, None]
    c_idx = np.arange(SC)[None, :]
    masks = np.ascontiguousarray(
        np.stack(
            [(r_idx <= c_idx - t * 128).astype(np.float32) for t in range(HPC)]
        )
    )
    ones = np.ones((128, 128), dtype=np.float32)
    # within each head, pack d-rows as [even dims; odd dims]
    perm = np.concatenate([np.arange(0, DK, 2), np.arange(1, DK, 2)])

    xT_b = [np.ascontiguousarray(x[b].T) for b in range(B)]

    in_maps = []
    for c in range(N_CORES):
        b = c // (N_CORES // B)
        g = c % (N_CORES // B)
        es = g * E

        def pack_qk(w):
            rows = w[es : es + E]  # [E, D]
            blocks = [
                rows[h0 * DK : (h0 + 1) * DK][perm] for h0 in range(HPC)
            ]
            return np.ascontiguousarray(np.concatenate(blocks, axis=0).T)

        def pack_bias(bvec):
            sl = bvec[es : es + E].reshape(HPC, DK)
            return np.ascontiguousarray(sl[:, perm])

        in_maps.append(
            {
                "xT": xT_b[b],
                "wqT": pack_qk(wq_w),
                "wkT": pack_qk(wk_w),
                "wvT": np.ascontiguousarray(wv_w[es : es + E].T),
                "woT": np.ascontiguousarray(wo_w[:, es : es + E].T),
                "bq": pack_bias(wq_b),
                "bk": pack_bias(wk_b),
                "cc2": cc2,
                "sss": sss,
                "masks": masks,
                "ones": ones,
            }
        )

    trace = bool(os.environ.get("MHA_TRACE"))
    res = run_bass_kernel_spmd(
        nc, in_maps, list(range(N_CORES)), trace=trace
    )
    last_exec_time_ns = res.exec_time_ns
    last_results = res

    # host-side gather: sum partials per batch, add biases that commute
    # with attention (softmax rows sum to 1, so wv_b passes straight
    # through to the output projection)
    const_bias = wo_b + wo_w @ wv_b  # [D]
    out = np.empty((B, S, D), dtype=np.float32)
    gpb = N_CORES // B
    for b in range(B):
        acc = res.results[b * gpb]["out"].copy()
        for c in range(b * gpb + 1, (b + 1) * gpb):
            acc += res.results[c]["out"]
        out[b] = acc + const_bias[None, :]
    return out



# revision 7
# speedup vs baseline: 1.0369x; 1.0369x over previous
"""Trainium2 Bass kernel for causal multi-head attention with RoPE.

Full-input contract: kernel(**inputs) takes the unsharded tensors and
returns the full [B, S, D] output. Internally the work is sharded over
8 NeuronCores: cores 0-3 compute batch 0, cores 4-7 batch 1; within a
batch group each core owns 4 of the 16 heads (tensor-parallel over
heads). Each core computes its partial output-projection contribution
[S, D]; the host sums the 4 partials per batch and adds the biases
that commute with attention (wo_b, and wv_b which passes through the
softmax untouched because attention weights sum to 1).

v2 layout: all operands fp16 (half the DMA/SBUF of fp32r at the same
PE rate), x/Q/K/V resident in SBUF (no DRAM bounce between phases),
V projected first, then per-head QK-projection + RoPE (RoPE on the
idle GpSimd engine) software-pipelined against the previous head's
attention so the exp latency hides under projection matmuls. Scores
exp in 2-PSUM-bank waves (one ACT instruction per 1024 columns), and
the softmax denominator comes from a fp16 DVE accumulation plus a
single ones-matmul per query chunk instead of a PE matmul per tile.
"""

import os
import sys

sys.path.insert(0, "/opt/trn_rl_repo")

import numpy as np

B = 2
S = 2048
D = 2048
H = 16
DK = 128
N_CORES = 8
HPC = 4          # heads per core
E = HPC * DK     # 512: per-core slice of the model dim
KO = D // 128    # contraction chunks for the projections
NJ = S // 128    # key blocks
SC = 512         # attention query chunk
NI = S // SC     # query chunks
ISQRT_DK = 1.0 / np.sqrt(DK)

_CACHE = {}

last_exec_time_ns = None
last_results = None


def _build_program():
    import concourse.mybir as mybir
    import concourse.tile as tile
    from concourse import bacc

    dt = mybir.dt
    F32 = dt.float32
    F16 = dt.float16
    AF = mybir.ActivationFunctionType
    ALU = mybir.AluOpType

    nc = bacc.Bacc(None, target_bir_lowering=False, debug=True)

    xT = nc.dram_tensor("xT", [D, S], F16, kind="ExternalInput")
    wqT = nc.dram_tensor("wqT", [D, E], F16, kind="ExternalInput")
    wkT = nc.dram_tensor("wkT", [D, E], F16, kind="ExternalInput")
    wvT = nc.dram_tensor("wvT", [D, E], F16, kind="ExternalInput")
    woT = nc.dram_tensor("woT", [E, D], F16, kind="ExternalInput")
    bq = nc.dram_tensor("bq", [HPC, DK], F32, kind="ExternalInput")
    bk = nc.dram_tensor("bk", [HPC, DK], F32, kind="ExternalInput")
    cc2 = nc.dram_tensor("cc2", [DK, S], F16, kind="ExternalInput")
    sss = nc.dram_tensor("sss", [DK, S], F16, kind="ExternalInput")
    masks = nc.dram_tensor("masks", [HPC, 128, SC], F16, kind="ExternalInput")
    ones = nc.dram_tensor("ones", [128, 128], F16, kind="ExternalInput")
    out = nc.dram_tensor("out", [S, D], F16, kind="ExternalOutput")

    QDMA = None  # round-robin queue set for bulk loads

    with tile.TileContext(nc) as tc:
        with (
            tc.tile_pool(name="const", bufs=1) as cpool,
            tc.tile_pool(name="res", bufs=1) as respool,
            tc.tile_pool(name="wqk", bufs=1) as wqkpool,
            tc.tile_pool(name="bwork", bufs=3) as p2pool,
            tc.tile_pool(name="bacc", bufs=2) as accpool,
            tc.tile_pool(name="bli", bufs=2) as lipool,
            tc.tile_pool(name="xres", bufs=1) as xpool,
        ):
            QDMA = [nc.sync, nc.scalar, nc.gpsimd, nc.sync]

            # ---- resident tiles ----
            x_sb = xpool.tile([128, KO, S], F16, name="x_sb")
            vt = respool.tile([128, NJ, E], F16, name="vt")
            qres = respool.tile([DK, HPC, S], F16, name="qres")
            kres = respool.tile([DK, HPC, S], F16, name="kres")
            aores = respool.tile([DK, HPC, S], F16, name="aores")
            wq_sb = wqkpool.tile([128, KO, E], F16, name="wq_sb")
            wk_sb = wqkpool.tile([128, KO, E], F16, name="wk_sb")

            # ---- input streams, need-ordered, spread over 4 queues ----
            # wv first (V runs first), then x chunk by chunk, then wq/wk.
            wv_ctx = tc.tile_pool(name="wv", bufs=1)
            wvpool = wv_ctx.__enter__()
            wv_sb = wvpool.tile([128, KO, E], F16, name="wv_sb")
            for g in range(4):
                QDMA[g].dma_start(
                    wv_sb[:, g * 4 : (g + 1) * 4, :],
                    wvT[g * 512 : (g + 1) * 512, :].rearrange(
                        "(ko p) m -> p ko m", p=128
                    ),
                )
            for si in range(4):
                sl = slice(si * 512, (si + 1) * 512)
                for g in range(4):
                    QDMA[g].dma_start(
                        x_sb[:, g * 4 : (g + 1) * 4, sl],
                        xT[g * 512 : (g + 1) * 512, sl].rearrange(
                            "(ko p) s -> p ko s", p=128
                        ),
                    )
            # small constants on the gpsimd queue (gpsimd compute starts late)
            bq_sb = cpool.tile([DK, HPC], F32, name="bq_sb")
            nc.gpsimd.dma_start(bq_sb[:], bq[:].rearrange("h d -> d h"))
            bk_sb = cpool.tile([DK, HPC], F32, name="bk_sb")
            nc.gpsimd.dma_start(bk_sb[:], bk[:].rearrange("h d -> d h"))
            cc2_sb = cpool.tile([DK, S], F16, name="cc2_sb")
            nc.gpsimd.dma_start(cc2_sb[:], cc2[:])
            sss_sb = cpool.tile([DK, S], F16, name="sss_sb")
            nc.gpsimd.dma_start(sss_sb[:], sss[:])
            mask_sb = cpool.tile([128, HPC, SC], F16, name="mask_sb")
            nc.gpsimd.dma_start(mask_sb[:], masks[:].rearrange("t p c -> p t c"))
            ones_sb = cpool.tile([128, 128], F16, name="ones_sb")
            nc.gpsimd.dma_start(ones_sb[:], ones[:])
            # weights for Q/K projections (needed after V completes)
            for wsb, wdram in ((wk_sb, wkT), (wq_sb, wqT)):
                for g in range(4):
                    QDMA[g].dma_start(
                        wsb[:, g * 4 : (g + 1) * 4, :],
                        wdram[g * 512 : (g + 1) * 512, :].rearrange(
                            "(ko p) m -> p ko m", p=128
                        ),
                    )

            # ---------- Phase V: value projection, vt resident ----------
            vps_ctx = tc.tile_pool(name="vps", bufs=2, space="PSUM")
            vpspool = vps_ctx.__enter__()
            for si in range(4):
                for jj in range(4):
                    pv = vpspool.tile([128, E], F32, tag="pv")
                    sl = slice(si * 512 + jj * 128, si * 512 + (jj + 1) * 128)
                    for k in range(KO):
                        nc.tensor.matmul(
                            pv[:],
                            x_sb[:, k, sl],
                            wv_sb[:, k, :],
                            start=(k == 0),
                            stop=(k == KO - 1),
                        )
                    nc.vector.tensor_copy(vt[:, si * 4 + jj, :], pv[:])
            vps_ctx.__exit__(None, None, None)
            wv_ctx.__exit__(None, None, None)

            # ---------- interleaved QK projection + attention ----------
            qkps_ctx = tc.tile_pool(name="qkps", bufs=2, space="PSUM")
            pqpool = qkps_ctx.__enter__()
            bps_ctx = tc.tile_pool(name="bps", bufs=2, space="PSUM")
            ps2pool = bps_ctx.__enter__()
            bpo_ctx = tc.tile_pool(name="bpo", bufs=1, space="PSUM")
            popool = bpo_ctx.__enter__()
            bpl_ctx = tc.tile_pool(name="bpl", bufs=1, space="PSUM")
            plpool = bpl_ctx.__enter__()
            st_ctx = tc.tile_pool(name="stw", bufs=2)
            stpool = st_ctx.__enter__()
            sw_ctx = tc.tile_pool(name="sww", bufs=1)
            swpool = sw_ctx.__enter__()

            # pre-zero the two score PSUM buffers: diagonal-band matmuls
            # write [cs:] only, and exp reads the full strip (the masked
            # region must be finite, not virgin-PSUM NaN)
            z0 = ps2pool.tile([128, 2, SC], F32, tag="ps2", name="z0")
            nc.vector.memset(z0[:], 0.0)
            z1 = ps2pool.tile([128, 2, SC], F32, tag="ps2", name="z1")
            nc.vector.memset(z1[:], 0.0)

            def qkproj_chunks(h):
                """8 emission chunks: (k, nsl0), (q, nsl0), (k, nsl1), ..."""
                chunks = []
                for nsl in range(4):
                    for wsb, bsb, dst in (
                        (wk_sb, bk_sb, kres),
                        (wq_sb, bq_sb, qres),
                    ):
                        def emit(nsl=nsl, wsb=wsb, bsb=bsb, dst=dst):
                            sl = slice(nsl * 512, (nsl + 1) * 512)
                            pq = pqpool.tile([128, SC], F32, tag="pq")
                            for k in range(KO):
                                nc.tensor.matmul(
                                    pq[:],
                                    wsb[:, k, h * DK : (h + 1) * DK],
                                    x_sb[:, k, sl],
                                    start=(k == 0),
                                    stop=(k == KO - 1),
                                )
                            st = stpool.tile([128, SC], F16, tag="st")
                            nc.scalar.activation(
                                st[:], pq[:], AF.Identity,
                                bias=bsb[:, h : h + 1],
                            )
                            # RoPE: d-rows packed [even; odd], so the rotate
                            # partner is partition r <-> r+64. Cross-partition
                            # swap on DVE, multiplies on the idle gpsimd.
                            sw = swpool.tile([128, SC], F16, tag="sw")
                            nc.vector.tensor_copy(sw[0:64, :], st[64:128, :])
                            nc.vector.tensor_copy(sw[64:128, :], st[0:64, :])
                            nc.gpsimd.tensor_mul(sw[:], sw[:], sss_sb[:, sl])
                            nc.gpsimd.tensor_mul(
                                dst[:, h, sl], st[:], cc2_sb[:, sl]
                            )
                            nc.gpsimd.tensor_add(
                                dst[:, h, sl], dst[:, h, sl], sw[:]
                            )
                        chunks.append(emit)
                return chunks

            def attn_chunks(h, p2pool=p2pool, accpool=accpool, lipool=lipool):
                """4 emission chunks, one per query chunk ic."""
                chunks = []
                for ic in range(NI):
                    def emit(ic=ic):
                        njc = 4 * ic + 4
                        i0 = ic * SC
                        po = popool.tile([128, SC], F32, tag="po")
                        acc = accpool.tile([128, SC], F16, tag="acc")
                        pend = []

                        def flush(wave, ws2):
                            # one exp over the whole 2-bank wave
                            nw = len(wave)
                            p2 = p2pool.tile([128, 2, SC], F16, tag="p2")
                            nc.scalar.activation(
                                p2[:, 0:nw, :], ws2[:, 0:nw, :], AF.Exp,
                                scale=float(ISQRT_DK),
                            )
                            for j, (jc, t, cs) in enumerate(wave):
                                if t >= 0:
                                    nc.vector.tensor_mul(
                                        p2[:, j, 0 : cs + 128],
                                        p2[:, j, 0 : cs + 128],
                                        mask_sb[:, t, 0 : cs + 128],
                                    )
                                if jc == 0:
                                    nc.vector.tensor_copy(acc[:], p2[:, j, :])
                                else:
                                    nc.vector.tensor_add(
                                        acc[:], acc[:], p2[:, j, :]
                                    )
                            pend.append((p2, wave))

                        def drain_pv():
                            p2, wave = pend.pop(0)
                            for j, (jc, t, cs) in enumerate(wave):
                                nc.tensor.matmul(
                                    po[:, cs:],
                                    vt[:, jc, h * DK : (h + 1) * DK],
                                    p2[:, j, cs:],
                                    start=(jc == 0),
                                    stop=(jc == njc - 1),
                                )

                        wave, ws2 = [], None
                        for jc in range(njc):
                            t = jc - 4 * ic
                            cs = 128 * t if t >= 0 else 0
                            if not wave:
                                ws2 = ps2pool.tile([128, 2, SC], F32, tag="ps2")
                            nc.tensor.matmul(
                                ws2[:, len(wave), cs:],
                                kres[:, h, jc * 128 : (jc + 1) * 128],
                                qres[:, h, i0 + cs : i0 + SC],
                                start=True,
                                stop=True,
                            )
                            wave.append((jc, t, cs))
                            if len(wave) == 2:
                                flush(wave, ws2)
                                wave, ws2 = [], None
                                if len(pend) > 1:
                                    drain_pv()
                        while pend:
                            drain_pv()

                        # denominator: one ones-matmul on the accumulated
                        # per-block sums, then reciprocal + normalize
                        pl = plpool.tile([128, SC], F32, tag="pl")
                        nc.tensor.matmul(
                            pl[:], ones_sb[:], acc[:], start=True, stop=True
                        )
                        li = lipool.tile([128, SC], F32, tag="li")
                        nc.vector.reciprocal_approx_fast(li[:], pl[:])
                        nc.vector.tensor_mul(
                            aores[:, h, i0 : i0 + SC], po[:], li[:]
                        )
                    chunks.append(emit)
                return chunks

            # schedule: qkproj(0), then per head: attention(h) with
            # qkproj(h+1) chunks slotted after each query chunk
            for ch in qkproj_chunks(0):
                ch()
            for h in range(HPC - 1):
                nxt = qkproj_chunks(h + 1)
                at = attn_chunks(h)
                for ic in range(NI):
                    at[ic]()
                    for ch in nxt[2 * ic : 2 * ic + 2]:
                        ch()

            # projection scratch is done; swap it for the output weights and
            # prefetch them during the last head's attention
            sw_ctx.__exit__(None, None, None)
            st_ctx.__exit__(None, None, None)
            wo_ctx = tc.tile_pool(name="wo", bufs=1)
            wopool = wo_ctx.__enter__()
            wo_sb = wopool.tile([128, HPC, D], F16, name="wo_sb")
            for g in range(4):
                (nc.sync if g % 2 == 0 else nc.gpsimd).dma_start(
                    wo_sb[:, g, :],
                    woT[g * 128 : (g + 1) * 128, :],
                )
            for ch in attn_chunks(HPC - 1):
                ch()

            bpl_ctx.__exit__(None, None, None)
            bpo_ctx.__exit__(None, None, None)
            bps_ctx.__exit__(None, None, None)
            qkps_ctx.__exit__(None, None, None)

            # ---------- Phase C: output projection (partial sums) ----------
            with (
                tc.tile_pool(name="cob", bufs=4) as obpool,
                tc.tile_pool(name="cps", bufs=2, space="PSUM") as cpspool,
            ):
                for ii in range(S // 128):
                    isl = slice(ii * 128, (ii + 1) * 128)
                    for half in range(2):
                        pc = cpspool.tile([128, 2, 512], F32, tag="pc")
                        for ec in range(HPC):
                            for f2 in range(2):
                                fc = half * 2 + f2
                                nc.tensor.matmul(
                                    pc[:, f2, :],
                                    aores[:, ec, isl],
                                    wo_sb[:, ec, fc * 512 : (fc + 1) * 512],
                                    start=(ec == 0),
                                    stop=(ec == HPC - 1),
                                )
                        for f2 in range(2):
                            fc = half * 2 + f2
                            ob = obpool.tile([128, 512], F16, tag="ob")
                            if f2 == 0:
                                nc.vector.tensor_copy(ob[:], pc[:, f2, :])
                            else:
                                nc.scalar.activation(
                                    ob[:], pc[:, f2, :], AF.Copy
                                )
                            nc.sync.dma_start(
                                out[isl, fc * 512 : (fc + 1) * 512], ob[:]
                            )
            wo_ctx.__exit__(None, None, None)

    nc.compile()
    return nc


def _rope_tables():
    inv_freq = 1.0 / (10000.0 ** (np.arange(0, DK, 2, dtype=np.float64) / DK))
    pos = np.arange(S, dtype=np.float64)
    freqs = pos[:, None] * inv_freq[None, :]  # [S, DK/2]
    cos_t = np.cos(freqs).T.astype(np.float16)  # [64, S]
    sin_t = np.sin(freqs).T.astype(np.float16)
    cc2 = np.ascontiguousarray(np.concatenate([cos_t, cos_t], axis=0))
    sss = np.ascontiguousarray(np.concatenate([-sin_t, sin_t], axis=0))
    return cc2, sss


def kernel(
    x, wq_w, wq_b, wk_w, wk_b, wv_w, wv_b, wo_w, wo_b
) -> np.ndarray:
    global last_exec_time_ns, last_results
    from concourse.bass_utils import run_bass_kernel_spmd

    if "nc" not in _CACHE:
        _CACHE["nc"] = _build_program()
    nc = _CACHE["nc"]

    x = np.asarray(x, dtype=np.float32)
    wq_w = np.asarray(wq_w, dtype=np.float32)
    wk_w = np.asarray(wk_w, dtype=np.float32)
    wv_w = np.asarray(wv_w, dtype=np.float32)
    wo_w = np.asarray(wo_w, dtype=np.float32)
    wq_b = np.asarray(wq_b, dtype=np.float32)
    wk_b = np.asarray(wk_b, dtype=np.float32)
    wv_b = np.asarray(wv_b, dtype=np.float32)
    wo_b = np.asarray(wo_b, dtype=np.float32)

    cc2, sss = _rope_tables()
    r_idx = np.arange(128)[:, None]
    c_idx = np.arange(SC)[None, :]
    masks = np.ascontiguousarray(
        np.stack(
            [(r_idx <= c_idx - t * 128).astype(np.float16) for t in range(HPC)]
        )
    )
    ones = np.ones((128, 128), dtype=np.float16)
    # within each head, pack d-rows as [even dims; odd dims]
    perm = np.concatenate([np.arange(0, DK, 2), np.arange(1, DK, 2)])

    xT_b = [np.ascontiguousarray(x[b].T.astype(np.float16)) for b in range(B)]

    in_maps = []
    for c in range(N_CORES):
        b = c // (N_CORES // B)
        g = c % (N_CORES // B)
        es = g * E

        def pack_qk(w):
            rows = w[es : es + E]  # [E, D]
            blocks = [
                rows[h0 * DK : (h0 + 1) * DK][perm] for h0 in range(HPC)
            ]
            return np.ascontiguousarray(
                np.concatenate(blocks, axis=0).T.astype(np.float16)
            )

        def pack_bias(bvec):
            sl = bvec[es : es + E].reshape(HPC, DK)
            return np.ascontiguousarray(sl[:, perm])

        in_maps.append(
            {
                "xT": xT_b[b],
                "wqT": pack_qk(wq_w),
                "wkT": pack_qk(wk_w),
                "wvT": np.ascontiguousarray(
                    wv_w[es : es + E].T.astype(np.float16)
                ),
                "woT": np.ascontiguousarray(
                    wo_w[:, es : es + E].T.astype(np.float16)
                ),
                "bq": pack_bias(wq_b),
                "bk": pack_bias(wk_b),
                "cc2": cc2,
                "sss": sss,
                "masks": masks,
                "ones": ones,
            }
        )

    trace = bool(os.environ.get("MHA_TRACE"))
    res = run_bass_kernel_spmd(
        nc, in_maps, list(range(N_CORES)), trace=trace
    )
    last_exec_time_ns = res.exec_time_ns
    last_results = res

    # host-side gather: sum partials per batch, add biases that commute
    # with attention (softmax rows sum to 1, so wv_b passes straight
    # through to the output projection)
    const_bias = wo_b + wo_w @ wv_b  # [D]
    out = np.empty((B, S, D), dtype=np.float32)
    gpb = N_CORES // B
    for b in range(B):
        acc = res.results[b * gpb]["out"].astype(np.float32)
        for c in range(b * gpb + 1, (b + 1) * gpb):
            acc += res.results[c]["out"].astype(np.float32)
        out[b] = acc + const_bias[None, :]
    return out
